# revision 1
# baseline (speedup 1.0000x reference)
"""Distributed FWHT (Hamiltonian -> Pauli-string coefficients) on 8 TRN2 cores.

Computes y = FWHT(x) / N for N = 2^24, sharded contiguously across 8 cores
(2^21 elements each).  FWHT = H8 (core axis) (x) H128 (x) H128 (x) H128.

Per-core kernel:
  - 3 local PE passes, each a "data-stationary" matmul sweep: 128 chunks of
    [128,128]; lhsT = data chunk, rhs = H128/128 -> out = chunk.T @ Hs.
    Each pass transforms the partition axis and rotates the layout so the
    next 7-bit axis lands on partitions.  All matmul reads are contiguous;
    the PSUM->SBUF copies write strided (runs of 4B @ 512B stride).
  - AllToAll across the 8 cores (chunked on the leading 7-bit axis).
  - One final PE pass with stationary kron(H8, I16)/8 over partitions
    (c', a'sub), combining the 8 cores' contributions.
Scaling by 1/2^24 is folded into the transform matrices (exact powers of 2).
"""

import math

import numpy as np

NCORES = 8
P = 128
F = 16384  # free elements per partition (2^21 per core / 128)
LOCAL = P * F


def _hadamard(n: int) -> np.ndarray:
    H = np.array([[1.0]], dtype=np.float64)
    while H.shape[0] < n:
        H = np.block([[H, H], [H, -H]])
    return H


_BUILD_CACHE: dict = {}


def _build_module():
    """Build + schedule the Bass module once per process."""
    if "nc" in _BUILD_CACHE:
        return _BUILD_CACHE["nc"]

    import concourse.bass as bass
    import concourse.mybir as mybir
    import concourse.tile as tile
    from concourse import bacc

    f32 = mybir.dt.float32

    Hs_np = (_hadamard(128) / 128.0).astype(np.float32)
    M_np = (np.kron(_hadamard(8), np.eye(16)) / 8.0).astype(np.float32)

    nc = bacc.Bacc(
        "TRN2",
        target_bir_lowering=False,
        debug=False,
        enable_asserts=False,
        num_devices=NCORES,
    )

    x_in = nc.dram_tensor("x", [P, F], f32, kind="ExternalInput")
    y_out = nc.dram_tensor("y", [P, F], f32, kind="ExternalOutput")
    Hs_dram = nc.inline_tensor(Hs_np, name="Hs_const")
    M_dram = nc.inline_tensor(M_np, name="M_const")

    with tile.TileContext(nc) as tc:
        with (
            tc.tile_pool(name="big", bufs=2) as big,
            tc.tile_pool(name="consts", bufs=1) as consts,
            tc.tile_pool(name="psum", bufs=6, space="PSUM") as psum,
            tc.tile_pool(name="dram", bufs=1, space="DRAM") as dram,
        ):
            Hs_t = consts.tile([P, 128], f32, tag="hs")
            M_t = consts.tile([P, 128], f32, tag="m")
            nc.sync.dma_start(Hs_t[:], Hs_dram[:])
            nc.sync.dma_start(M_t[:], M_dram[:])

            a2a_in = dram.tile([P, F], f32, tag="a2a_in")
            a2a_out = dram.tile([P, F], f32, tag="a2a_out")

            X = big.tile([P, F], f32, tag="big")
            # load input in 4 column blocks so pass 1 can start early
            for k in range(4):
                nc.sync.dma_start(
                    X[:, k * 4096 : (k + 1) * 4096], x_in[:, k * 4096 : (k + 1) * 4096]
                )

            def pass_sweep(src, dst, stationary):
                """One data-stationary FWHT pass: 128 chunk matmuls + copies.

                src layout [p; (u, v)]; chunk i = src[:, 128i:128i+128];
                out[v'; p'] written to dst cols {p'*128 + i} (dst [v'?]...
                layout [chunk-free; (p', i)]).
                """
                dst_r = dst[:].rearrange("p (a b) -> p b a", b=128)
                for g in range(32):
                    pt = psum.tile([P, 512], f32, tag="ps")
                    for j in range(4):
                        i = g * 4 + j
                        nc.tensor.matmul(
                            pt[:, j * 128 : (j + 1) * 128],
                            src[:, i * 128 : (i + 1) * 128],
                            stationary[:],
                        )
                    eng = nc.vector.tensor_copy if g % 2 == 0 else nc.scalar.copy
                    eng(
                        dst_r[:, g * 4 : (g + 1) * 4, :],
                        pt[:].rearrange("p (j a) -> p j a", j=4),
                    )

            Y = big.tile([P, F], f32, tag="big")
            pass_sweep(X, Y, Hs_t)
            Z = big.tile([P, F], f32, tag="big")
            pass_sweep(Y, Z, Hs_t)
            W = big.tile([P, F], f32, tag="big")
            pass_sweep(Z, W, Hs_t)

            nc.sync.dma_start(a2a_in[:], W[:])
            nc.gpsimd.collective_compute(
                "AllToAll",
                mybir.AluOpType.bypass,
                replica_groups=[list(range(NCORES))],
                ins=[a2a_in.opt()],
                outs=[a2a_out.opt()],
            )

            V = big.tile([P, F], f32, tag="big")
            nc.sync.dma_start(V[:], a2a_out[:])

            O = big.tile([P, F], f32, tag="big")
            for g in range(32):
                pt = psum.tile([P, 512], f32, tag="ps")
                nc.tensor.matmul(pt[:], M_t[:], V[:, g * 512 : (g + 1) * 512])
                eng = nc.vector.tensor_copy if g % 2 == 0 else nc.scalar.copy
                eng(O[:, g * 512 : (g + 1) * 512], pt[:])

            nc.sync.dma_start(y_out[:], O[:])

    nc.compile()
    _BUILD_CACHE["nc"] = nc
    return nc


def run(x: np.ndarray, trace: bool = False):
    """Run the 8-core kernel on the full input vector.

    Returns (y_full, BassKernelResults)."""
    from concourse.bass_utils import run_bass_kernel_spmd

    nc = _build_module()
    x = np.ascontiguousarray(x, dtype=np.float32)
    assert x.shape == (NCORES * LOCAL,)
    shards = x.reshape(NCORES, P, F)
    in_maps = [{"x": shards[c]} for c in range(NCORES)]
    res = run_bass_kernel_spmd(
        nc, in_maps, core_ids=list(range(NCORES)), trace=trace
    )
    # gather: y[c*2^21 + (16q+s)*2^14 + f] = O_q[c*16+s, f]
    outs = [res.results[q]["y"].reshape(NCORES, 16, F) for q in range(NCORES)]
    full = np.stack(outs, axis=1)  # (c, q, s, f)
    return full.reshape(NCORES * LOCAL), res


def kernel(Hamiltonian: np.ndarray) -> np.ndarray:
    y, _ = run(Hamiltonian, trace=False)
    return y



# revision 3
# speedup vs baseline: 1.0305x; 1.0305x over previous
"""Distributed FWHT (Hamiltonian -> Pauli-string coefficients) on 8 TRN2 cores.

v3 vs v2:
  - PSUM->SBUF copies split across THREE engines (DVE, Act, Pool) with
    1024-wide copies ([128,1024] 2-bank PSUM tiles, bufs=4) to halve the
    per-pass drain wall (the pre-collective bottleneck).
  - Collectives issued from SP (sync) so their input-waits don't stall the
    Pool copy stream; a2a_in DMAs triggered from DVE; V loads + y stores on
    Pool (idle after pass 3).
  - M-pass copies on DVE+Act only (Pool handles V/y DMAs then).

Math/layout identical to v2 (see v2 docstring): 3 data-stationary bf16 H128
passes (scatter, scatter, contiguous), chunked AllToAll over 16-row groups,
final kron(H8,I16)/8 combine, all scaling folded in, bf16 end-to-end.
"""

import math

import numpy as np
import ml_dtypes

NCORES = 8
P = 128
F = 16384
LOCAL = P * F
K = 2  # a2a chunks
FK = F // K
G = 1024  # copy width (2 PSUM banks)
NG = F // G  # 16 copy groups per pass
GPK = NG // K  # copy groups per a2a chunk

# per-pass engine assignment for the 16 copy groups.  GPSIMD cannot touch
# PSUM (real-compiler restriction), so only DVE (0.96GHz) and Act (1.2GHz)
# can drain; Act gets the larger share.
COPY_ENG = [
    "dve", "act", "dve", "act", "dve", "act", "dve", "act",
    "act", "act", "dve", "act", "dve", "act", "dve", "act",
]


def _hadamard(n: int) -> np.ndarray:
    H = np.array([[1.0]], dtype=np.float64)
    while H.shape[0] < n:
        H = np.block([[H, H], [H, -H]])
    return H


_BUILD_CACHE: dict = {}


def _build_module():
    if "nc" in _BUILD_CACHE:
        return _BUILD_CACHE["nc"]

    import concourse.bass as bass
    import concourse.mybir as mybir
    import concourse.tile as tile
    from concourse import bacc

    f32 = mybir.dt.float32
    bf16 = mybir.dt.bfloat16

    Hs_np = (_hadamard(128) / 128.0).astype(ml_dtypes.bfloat16)
    M_np = (np.kron(_hadamard(8), np.eye(16)) / 8.0).astype(ml_dtypes.bfloat16)

    nc = bacc.Bacc(
        "TRN2",
        target_bir_lowering=False,
        debug=False,
        enable_asserts=False,
        num_devices=NCORES,
    )

    x_in = nc.dram_tensor("x", [P, F], bf16, kind="ExternalInput")
    y_out = nc.dram_tensor("y", [P, F], bf16, kind="ExternalOutput")
    Hs_dram = nc.inline_tensor(Hs_np, name="Hs_const")
    M_dram = nc.inline_tensor(M_np, name="M_const")

    with tile.TileContext(nc) as tc:
        with (
            tc.tile_pool(name="big", bufs=2) as big,
            tc.tile_pool(name="vo", bufs=2) as vo,
            tc.tile_pool(name="consts", bufs=1) as consts,
            tc.tile_pool(name="psum", bufs=4, space="PSUM") as psum,
            tc.tile_pool(name="dram", bufs=1, space="DRAM") as dram,
        ):
            Hs_t = consts.tile([P, 128], bf16, tag="hs")
            M_t = consts.tile([P, 128], bf16, tag="m")
            nc.sync.dma_start(Hs_t[:], Hs_dram[:])
            nc.sync.dma_start(M_t[:], M_dram[:])

            a2a_in = [
                dram.tile([P, FK], bf16, tag=f"a2a_in{k}", name=f"a2a_in{k}")
                for k in range(K)
            ]
            a2a_out = [
                dram.tile([P, FK], bf16, tag=f"a2a_out{k}", name=f"a2a_out{k}")
                for k in range(K)
            ]

            def copy_eng(g):
                return {
                    "dve": nc.vector.tensor_copy,
                    "act": nc.scalar.copy,
                    "pool": nc.gpsimd.tensor_copy,
                }[COPY_ENG[g]]

            X = big.tile([P, F], bf16, tag="big")
            for k in range(8):
                nc.sync.dma_start(
                    X[:, k * 2048 : (k + 1) * 2048], x_in[:, k * 2048 : (k + 1) * 2048]
                )

            def pass_scatter(src, dst):
                """Data-stationary FWHT pass with strided (rotating) writes."""
                dst_r = dst[:].rearrange("p (a b) -> p b a", b=128)
                for g in range(NG):
                    pt = psum.tile([P, G], f32, tag="ps")
                    for j in range(8):
                        i = g * 8 + j
                        nc.tensor.matmul(
                            pt[:, j * 128 : (j + 1) * 128],
                            src[:, i * 128 : (i + 1) * 128],
                            Hs_t[:],
                        )
                    copy_eng(g)(
                        dst_r[:, g * 8 : (g + 1) * 8, :],
                        pt[:].rearrange("p (j a) -> p j a", j=8),
                    )

            Y = big.tile([P, F], bf16, tag="big")
            pass_scatter(X, Y)
            Z = big.tile([P, F], bf16, tag="big")
            pass_scatter(Y, Z)

            # pass 3: contiguous writes; ship each a2a chunk as it completes
            W = big.tile([P, F], bf16, tag="big")
            for g in range(NG):
                pt = psum.tile([P, G], f32, tag="ps")
                for j in range(8):
                    i = g * 8 + j
                    nc.tensor.matmul(
                        pt[:, j * 128 : (j + 1) * 128],
                        Z[:, i * 128 : (i + 1) * 128],
                        Hs_t[:],
                    )
                copy_eng(g)(W[:, g * G : (g + 1) * G], pt[:])
                # fill a2a_in progressively in 2-group (0.5MB) pieces on
                # alternating queues so the collective launch isn't gated on
                # one long DMA
                if g % 2 == 1:
                    k = g // GPK
                    c0 = (g - 1) * G
                    deng = nc.sync if (g // 2) % 2 == 0 else nc.gpsimd
                    deng.dma_start(
                        a2a_in[k][:, c0 - k * FK : c0 - k * FK + 2 * G],
                        W[:, c0 : c0 + 2 * G],
                    )
                if g % GPK == GPK - 1:
                    k = g // GPK
                    nc.gpsimd.collective_compute(
                        "AllToAll",
                        mybir.AluOpType.bypass,
                        replica_groups=[list(range(NCORES))],
                        ins=[a2a_in[k].opt()],
                        outs=[a2a_out[k].opt()],
                    )

            # combine pass per chunk: O = kron(H8,I16)/8 . V, then store
            for k in range(K):
                V = vo.tile([P, FK], bf16, tag="vo")
                for b in range(8):
                    nc.sync.dma_start(
                        V[:, b * 1024 : (b + 1) * 1024],
                        a2a_out[k][:, b * 1024 : (b + 1) * 1024],
                    )
                O = vo.tile([P, FK], bf16, tag="vo")
                for g in range(FK // G):
                    pt = psum.tile([P, G], f32, tag="ps")
                    for j in range(2):
                        c0 = g * G + j * 512
                        nc.tensor.matmul(
                            pt[:, j * 512 : (j + 1) * 512],
                            M_t[:],
                            V[:, c0 : c0 + 512],
                        )
                    eng = nc.vector.tensor_copy if g % 2 == 0 else nc.scalar.copy
                    eng(O[:, g * G : (g + 1) * G], pt[:])
                    if g % 2 == 1:
                        c0 = k * FK + (g - 1) * G
                        nc.sync.dma_start(
                            y_out[:, c0 : c0 + 2 * G],
                            O[:, (g - 1) * G : (g + 1) * G],
                        )

    nc.compile()
    _BUILD_CACHE["nc"] = nc
    return nc


def run(x: np.ndarray, trace: bool = False):
    from concourse.bass_utils import run_bass_kernel_spmd

    nc = _build_module()
    x = np.ascontiguousarray(x, dtype=np.float32).astype(ml_dtypes.bfloat16)
    assert x.shape == (NCORES * LOCAL,)
    shards = x.reshape(NCORES, P, F)
    in_maps = [{"x": shards[c]} for c in range(NCORES)]
    res = run_bass_kernel_spmd(nc, in_maps, core_ids=list(range(NCORES)), trace=trace)
    # gather: y[c'*2^21 + (16q+s)*2^14 + b*2^7 + c] = O_q[c'*16+s, c*128+b]
    outs = [
        res.results[q]["y"].astype(np.float32).reshape(NCORES, 16, 128, 128)
        for q in range(NCORES)
    ]
    full = np.stack(outs, axis=1)  # (c', q, s, chat, bhat)
    full = np.transpose(full, (0, 1, 2, 4, 3))
    return np.ascontiguousarray(full).reshape(NCORES * LOCAL), res


def kernel(Hamiltonian: np.ndarray) -> np.ndarray:
    y, _ = run(Hamiltonian, trace=False)
    return y


# revision 6
# speedup vs baseline: 1.1746x; 1.1399x over previous
"""Distributed FWHT (Hamiltonian -> Pauli-string coefficients) on 8 TRN2 cores.

v3 vs v2:
  - PSUM->SBUF copies split across THREE engines (DVE, Act, Pool) with
    1024-wide copies ([128,1024] 2-bank PSUM tiles, bufs=4) to halve the
    per-pass drain wall (the pre-collective bottleneck).
  - Collectives issued from SP (sync) so their input-waits don't stall the
    Pool copy stream; a2a_in DMAs triggered from DVE; V loads + y stores on
    Pool (idle after pass 3).
  - M-pass copies on DVE+Act only (Pool handles V/y DMAs then).

Math/layout identical to v2 (see v2 docstring): 3 data-stationary bf16 H128
passes (scatter, scatter, contiguous), chunked AllToAll over 16-row groups,
final kron(H8,I16)/8 combine, all scaling folded in, bf16 end-to-end.
"""

import math

import numpy as np
import ml_dtypes

NCORES = 8
P = 128
F = 16384
LOCAL = P * F
G = 1024  # copy width (2 PSUM banks)
NG = F // G  # 16 copy groups per pass
# asymmetric a2a chunks (in units of G columns): a big first chunk that ships
# while pass 3 finishes, and a small second chunk so the post-collective tail
# (V load + M-pass + store) is tiny.
CHUNKS = [8, 8]
COFF = [0, 8]  # prefix offsets, groups

# per-pass engine assignment for the 16 copy groups.  GPSIMD cannot touch
# PSUM (real-compiler restriction), so only DVE (0.96GHz) and Act (1.2GHz)
# can drain; Act gets the larger share.
COPY_ENG = [
    "dve", "act", "dve", "act", "dve", "act", "dve", "act",
    "act", "act", "dve", "act", "dve", "act", "dve", "act",
]


def _hadamard(n: int) -> np.ndarray:
    H = np.array([[1.0]], dtype=np.float64)
    while H.shape[0] < n:
        H = np.block([[H, H], [H, -H]])
    return H


_BUILD_CACHE: dict = {}


def _build_module():
    if "nc" in _BUILD_CACHE:
        return _BUILD_CACHE["nc"]

    import concourse.bass as bass
    import concourse.mybir as mybir
    import concourse.tile as tile
    from concourse import bacc

    f32 = mybir.dt.float32
    bf16 = mybir.dt.bfloat16

    Hs_np = (_hadamard(128) / 128.0).astype(ml_dtypes.bfloat16)
    M_np = (np.kron(_hadamard(8), np.eye(16)) / 8.0).astype(ml_dtypes.bfloat16)

    nc = bacc.Bacc(
        "TRN2",
        target_bir_lowering=False,
        debug=False,
        enable_asserts=False,
        num_devices=NCORES,
    )

    x_in = nc.dram_tensor("x", [P, F], bf16, kind="ExternalInput")
    y_out = nc.dram_tensor("y", [P, F], bf16, kind="ExternalOutput")
    Hs_dram = nc.inline_tensor(Hs_np, name="Hs_const")
    M_dram = nc.inline_tensor(M_np, name="M_const")

    with tile.TileContext(nc) as tc:
        with (
            tc.tile_pool(name="big", bufs=2) as big,
            tc.tile_pool(name="vo", bufs=2) as vo,
            tc.tile_pool(name="consts", bufs=1) as consts,
            tc.tile_pool(name="psum", bufs=4, space="PSUM") as psum,
            tc.tile_pool(name="dram", bufs=1, space="DRAM") as dram,
        ):
            Hs_t = consts.tile([P, 128], bf16, tag="hs")
            M_t = consts.tile([P, 128], bf16, tag="m")
            nc.sync.dma_start(Hs_t[:], Hs_dram[:])
            nc.sync.dma_start(M_t[:], M_dram[:])

            a2a_in = [
                dram.tile([P, n * G], bf16, tag=f"a2a_in{k}", name=f"a2a_in{k}")
                for k, n in enumerate(CHUNKS)
            ]
            a2a_out = [
                dram.tile([P, n * G], bf16, tag=f"a2a_out{k}", name=f"a2a_out{k}")
                for k, n in enumerate(CHUNKS)
            ]

            def copy_eng(g):
                return {
                    "dve": nc.vector.tensor_copy,
                    "act": nc.scalar.copy,
                    "pool": nc.gpsimd.tensor_copy,
                }[COPY_ENG[g]]

            X = big.tile([P, F], bf16, tag="big")
            for k in range(8):
                nc.sync.dma_start(
                    X[:, k * 2048 : (k + 1) * 2048], x_in[:, k * 2048 : (k + 1) * 2048]
                )

            def pass_scatter(src, dst):
                """Data-stationary FWHT pass with strided (rotating) writes."""
                dst_r = dst[:].rearrange("p (a b) -> p b a", b=128)
                for g in range(NG):
                    pt = psum.tile([P, G], f32, tag="ps")
                    for j in range(8):
                        i = g * 8 + j
                        nc.tensor.matmul(
                            pt[:, j * 128 : (j + 1) * 128],
                            src[:, i * 128 : (i + 1) * 128],
                            Hs_t[:],
                        )
                    copy_eng(g)(
                        dst_r[:, g * 8 : (g + 1) * 8, :],
                        pt[:].rearrange("p (j a) -> p j a", j=8),
                    )

            Y = big.tile([P, F], bf16, tag="big")
            pass_scatter(X, Y)
            Z = big.tile([P, F], bf16, tag="big")
            pass_scatter(Y, Z)

            # pass 3: contiguous writes; ship each a2a chunk as it completes
            W = big.tile([P, F], bf16, tag="big")
            for g in range(NG):
                pt = psum.tile([P, G], f32, tag="ps")
                for j in range(8):
                    i = g * 8 + j
                    nc.tensor.matmul(
                        pt[:, j * 128 : (j + 1) * 128],
                        Z[:, i * 128 : (i + 1) * 128],
                        Hs_t[:],
                    )
                copy_eng(g)(W[:, g * G : (g + 1) * G], pt[:])
                # fill a2a_in progressively in 2-group (0.5MB) pieces on
                # alternating queues so the collective launch isn't gated on
                # one long DMA; launch each chunk's collective after its last
                # fill
                k = 0 if g < COFF[1] else 1
                lo = g - (g - COFF[k]) % 2  # start of this 2-group fill piece
                if g == lo + 1 or g == COFF[k] + CHUNKS[k] - 1:
                    w0 = lo * G
                    deng = nc.sync if (g // 2) % 2 == 0 else nc.gpsimd
                    deng.dma_start(
                        a2a_in[k][:, w0 - COFF[k] * G : (g + 1 - COFF[k]) * G],
                        W[:, w0 : (g + 1) * G],
                    )
                if g == COFF[k] + CHUNKS[k] - 1:
                    nc.gpsimd.collective_compute(
                        "AllToAll",
                        mybir.AluOpType.bypass,
                        replica_groups=[list(range(NCORES))],
                        ins=[a2a_in[k].opt()],
                        outs=[a2a_out[k].opt()],
                    )

            # combine pass per chunk: O = kron(H8,I16)/8 . V, then store
            for k, nk in enumerate(CHUNKS):
                FK = nk * G
                V = vo.tile([P, FK], bf16, tag=f"vo{k}", name=f"V{k}")
                for b in range(FK // 1024):
                    nc.sync.dma_start(
                        V[:, b * 1024 : (b + 1) * 1024],
                        a2a_out[k][:, b * 1024 : (b + 1) * 1024],
                    )
                O = vo.tile([P, FK], bf16, tag=f"vo{k}", name=f"O{k}")
                for g in range(nk):
                    pt = psum.tile([P, G], f32, tag="ps")
                    for j in range(2):
                        c0 = g * G + j * 512
                        nc.tensor.matmul(
                            pt[:, j * 512 : (j + 1) * 512],
                            M_t[:],
                            V[:, c0 : c0 + 512],
                        )
                    eng = nc.vector.tensor_copy if g % 2 == 0 else nc.scalar.copy
                    eng(O[:, g * G : (g + 1) * G], pt[:])
                    if g % 2 == 1:
                        c0 = (COFF[k] + g - 1) * G
                        nc.sync.dma_start(
                            y_out[:, c0 : c0 + 2 * G],
                            O[:, (g - 1) * G : (g + 1) * G],
                        )

    nc.compile()
    _BUILD_CACHE["nc"] = nc
    return nc


def run(x: np.ndarray, trace: bool = False):
    from concourse.bass_utils import run_bass_kernel_spmd

    nc = _build_module()
    x = np.ascontiguousarray(x, dtype=np.float32).astype(ml_dtypes.bfloat16)
    assert x.shape == (NCORES * LOCAL,)
    shards = x.reshape(NCORES, P, F)
    in_maps = [{"x": shards[c]} for c in range(NCORES)]
    res = run_bass_kernel_spmd(nc, in_maps, core_ids=list(range(NCORES)), trace=trace)
    # gather: y[c'*2^21 + (16q+s)*2^14 + b*2^7 + c] = O_q[c'*16+s, c*128+b]
    outs = [
        res.results[q]["y"].astype(np.float32).reshape(NCORES, 16, 128, 128)
        for q in range(NCORES)
    ]
    full = np.stack(outs, axis=1)  # (c', q, s, chat, bhat)
    full = np.transpose(full, (0, 1, 2, 4, 3))
    return np.ascontiguousarray(full).reshape(NCORES * LOCAL), res


def kernel(Hamiltonian: np.ndarray) -> np.ndarray:
    y, _ = run(Hamiltonian, trace=False)
    return y
